# revision 8
# baseline (speedup 1.0000x reference)
"""CP-ALS hash layer kernel for Trainium2 (8 NeuronCores, SPMD data-parallel).

Per sample: rank-32 CP-ALS (20 iters) on its (128,56,56) tensor; ridge-regularized
32x32 solves via Newton-Schulz (5 iters, Jacobi diag init); feats -> MLP -> sign
(MLP head on host, fp32). Batch 128 = 16 samples/core, processed in groups of 4
with factor-stacked (4x32=128 partition) DVE ops and tile_position-packed matmuls.

Wall-clock is dominated by shipping inputs over the ~40 MB/s axon tunnel, so:
  - x is sent as 24-bit fixed point (int16 MSB + uint8 LSB = 3 B/elem vs 4) and
    decoded to fp32 on device with exact power-of-two scales (the only loss is
    the initial rint(x * 2^20), ~3e-7 rms -- logits relerr ~2e-6, no sign flips).
  - the jitted shard_map executable is cached across calls (rebuilding it costs
    ~4 s of retrace per call inside run_bass_kernel_spmd's axon path).
  - konst is embedded in the NEFF (Const tensor); b0t/c0t initial factor
    transposes are computed on device instead of being shipped.

PSUM budget (8 banks of 2KB):
  ns   (1): grams gb/gc/ga/gb2 + NS s/xp slices
  u1   (1): M_A^T acc [0:128] | a_ps [128:256] | b_ps [256:384] | bt_ps [384:440]
  u2   (1): c_ps [0:128] | ct_ps [128:184]
  g    (2): G chunk double-buffer
  tp   (2): PE-transpose staging (T^T and P chunks; initial bt/ct transposes)
"""
import sys
sys.path.insert(0, '/opt/trn_rl_repo')
import numpy as np
from contextlib import ExitStack
from concurrent.futures import ThreadPoolExecutor

import concourse.bass as bass
import concourse.tile as tile
from concourse import bacc, mybir

F32 = mybir.dt.float32
I16 = mybir.dt.int16
U8 = mybir.dt.uint8

BSZ, CI, H, W = 128, 128, 56, 56
R = 32
N_ITERS = 20
RIDGE = 1e-6
NCORES = 8
SPC = BSZ // NCORES          # 16 samples per core
JK = H * W                   # 3136
JKP = 3200                   # JK padded to 25*128
NCHUNK = JKP // 128          # 25
GCH = [504] * 6 + [112]      # G chunks at j boundaries (9j*56 ... 2j*56)
NS_ITERS = 5

# 24-bit fixed point encode of x: xi = rint(x * 2^20) clipped to +-(2^23-1),
# hi = floor(xi/256) in int16, lo = xi - 256*hi in uint8.
# Decode: x ~= hi * 2^-12 + lo * 2^-20 (exact fp32 arithmetic).
S_ENC = float(2 ** 20)
CLIP = float(2 ** 23 - 1)
SC_HI = float(2.0 ** -12)
SC_LO = float(2.0 ** -20)
NCHK = 4                     # x ships as 4 chunk pairs to overlap encode+wire

_CACHE = {}


def _konst_blob():
    k = np.zeros((128, 225), dtype=np.float32)
    k[:, 0:128] = np.eye(128, dtype=np.float32)
    k[:, 128] = 1.0
    i32 = np.eye(R, dtype=np.float32)
    for u in range(4):
        k[32 * u:32 * u + 32, 129:161] = RIDGE * i32
        k[32 * u:32 * u + 32, 161:193] = 2.0 * i32
        k[32 * u:32 * u + 32, 193:225] = i32
    return k


def _build_program(n_groups=SPC // 4, n_iters=N_ITERS, ns_iters=NS_ITERS):
    nc = bacc.Bacc(None, target_bir_lowering=False)
    nsamp = 4 * n_groups

    # x arrives as NCHK chunk pairs (samples [4k, 4k+4) per core in chunk k) so
    # the host can overlap encoding chunk k+1 with streaming chunk k.
    d_xhi, d_xlo = [], []
    for k in range(NCHK):
        d_xhi.append(nc.declare_dram_parameter(
            f"xhi{k}", [nsamp // NCHK, CI, JK], I16, isOutput=False))
        d_xlo.append(nc.declare_dram_parameter(
            f"xlo{k}", [nsamp // NCHK, CI, JK], U8, isOutput=False))
    d_a0 = nc.declare_dram_parameter("a0", [nsamp, CI, R], F32, isOutput=False)
    d_b0 = nc.declare_dram_parameter("b0", [nsamp, H, R], F32, isOutput=False)
    d_c0 = nc.declare_dram_parameter("c0", [nsamp, W, R], F32, isOutput=False)
    d_out = nc.declare_dram_parameter("feats", [R, nsamp * 3], F32, isOutput=True)
    d_k = nc.inline_tensor(_konst_blob(), name="konst")
    d_xf = nc.dram_tensor("xf", [nsamp, CI, JK], F32)

    with ExitStack() as ctx:
        tc = ctx.enter_context(tile.TileContext(nc))

        # ---- phase 1: decode int24 -> fp32 scratch in DRAM ----
        rows = nsamp // NCHK
        with tc.tile_pool(name="dec", bufs=2) as dpool:
            for s in range(nsamp):
                hi_sb = dpool.tile([CI, JK], I16, tag="hi")
                lo_sb = dpool.tile([CI, JK], U8, tag="lo")
                nc.sync.dma_start(hi_sb[:], d_xhi[s // rows][s % rows])
                nc.sync.dma_start(lo_sb[:], d_xlo[s // rows][s % rows])
                dec = dpool.tile([CI, JK], F32, tag="dc")
                lo_f = dpool.tile([CI, JK], F32, tag="lf")
                nc.vector.tensor_scalar_mul(dec[:], hi_sb[:], SC_HI)
                nc.vector.tensor_scalar_mul(lo_f[:], lo_sb[:], SC_LO)
                nc.vector.tensor_add(dec[:], dec[:], lo_f[:])
                nc.sync.dma_start(d_xf[s], dec[:])

        konst = ctx.enter_context(tc.tile_pool(name="konst", bufs=1))
        tn_pool = ctx.enter_context(tc.tile_pool(name="tn", bufs=4))
        tt_pool = ctx.enter_context(tc.tile_pool(name="tt", bufs=4))
        small = ctx.enter_context(tc.tile_pool(name="small", bufs=2))
        fac = ctx.enter_context(tc.tile_pool(name="fac", bufs=2))
        big = ctx.enter_context(tc.tile_pool(name="big", bufs=1))
        pp_pool = ctx.enter_context(tc.tile_pool(name="ppool", bufs=2))
        ps1 = ctx.enter_context(tc.tile_pool(name="ps1", bufs=1, space="PSUM"))
        psN = ctx.enter_context(tc.tile_pool(name="psN", bufs=1, space="PSUM"))
        psG = ctx.enter_context(tc.tile_pool(name="psG", bufs=2, space="PSUM"))
        psT = ctx.enter_context(tc.tile_pool(name="psT", bufs=2, space="PSUM"))
        ptp = ctx.enter_context(tc.tile_pool(name="ptp", bufs=2))
        out_pool = ctx.enter_context(tc.tile_pool(name="outp", bufs=1))

        k_sb = konst.tile([128, 225], F32)
        nc.sync.dma_start(k_sb[:], d_k[:])
        ident = k_sb[:, 0:128]
        ones = k_sb[:, 128:129]
        twoI4 = k_sb[:, 161:193]
        i32x4 = k_sb[:, 193:225]

        out_sb = out_pool.tile([R, nsamp * 3], F32)

        for g in range(n_groups):
            # ---- load tensor + transpose copies ----
            tn = [tn_pool.tile([CI, JKP], F32, tag="tn", name=f"tn{g}_{u}") for u in range(4)]
            tt = [tt_pool.tile([128, JKP], F32, tag="tt", name=f"tt{g}_{u}") for u in range(4)]
            for u in range(4):
                nc.sync.dma_start(tn[u][:, 0:JK], d_xf[4 * g + u])
                nc.vector.memset(tn[u][:, JK:JKP], 0.0)
            for u in range(4):
                for c0 in range(0, NCHUNK, 4):
                    cs = list(range(c0, min(c0 + 4, NCHUNK)))
                    tp_ps = psT.tile([128, 512], F32, tag="tp")
                    for i, c in enumerate(cs):
                        nc.tensor.transpose(tp_ps[:, 128 * i:128 * i + 128],
                                            tn[u][:, 128 * c:128 * c + 128], ident)
                    nc.scalar.copy(tt[u][:, 128 * cs[0]:128 * cs[0] + 128 * len(cs)],
                                   tp_ps[:, 0:128 * len(cs)])

            # ---- factors ----
            a4 = fac.tile([CI, 128], F32, tag="a4")
            b4 = fac.tile([128, 128], F32, tag="b4")
            c4 = fac.tile([128, 128], F32, tag="c4")
            bt4 = fac.tile([128, H], F32, tag="bt4")
            ct4 = fac.tile([128, W], F32, tag="ct4")
            nc.vector.memset(b4[:], 0.0)
            nc.vector.memset(c4[:], 0.0)
            for u in range(4):
                nc.sync.dma_start(a4[:, 32 * u:32 * u + 32], d_a0[4 * g + u])
                nc.sync.dma_start(b4[0:H, 32 * u:32 * u + 32], d_b0[4 * g + u])
                nc.sync.dma_start(c4[0:W, 32 * u:32 * u + 32], d_c0[4 * g + u])
            # initial bt4/ct4 = b4^T[:, :H], c4^T[:, :W] via PE transpose
            tp_ps = psT.tile([128, 512], F32, tag="tp", name=f"bt0_{g}")
            nc.tensor.transpose(tp_ps[:, 0:128], b4[:], ident)
            nc.tensor.transpose(tp_ps[:, 128:256], c4[:], ident)
            nc.scalar.copy(bt4[:], tp_ps[:, 0:H])
            nc.scalar.copy(ct4[:], tp_ps[:, 128:128 + W])

            def grams(ns_t, col, mat, np_, tag):
                for u in range(4):
                    nc.tensor.matmul(ns_t[32 * u:32 * u + 32, col:col + 32],
                                     mat[:, 32 * u:32 * u + 32],
                                     mat[:, 32 * u:32 * u + 32],
                                     start=True, stop=True, tile_position=(0, 32 * u))
                g_sb = small.tile([128, R], F32, tag=tag, name="gr_" + tag)
                nc.scalar.copy(g_sb[:], ns_t[:, col:col + 32])
                return g_sb

            def ns_solve(ns_t, gx_sb, gy_sb, tag):
                s_t = psN.tile([128, 64], F32, tag="nss", name="nss_" + tag)
                v_sb = small.tile([128, R], F32, tag=tag + "v")
                nc.vector.tensor_mul(v_sb[:], gx_sb[:], gy_sb[:])
                dm = small.tile([128, R], F32, tag=tag + "dm")
                nc.vector.tensor_mul(dm[:], v_sb[:], i32x4)
                dcol = small.tile([128, 1], F32, tag=tag + "dc")
                nc.vector.reduce_sum(dcol[:], dm[:], axis=mybir.AxisListType.X)
                rd = small.tile([128, 1], F32, tag=tag + "rd")
                nc.vector.reciprocal(rd[:], dcol[:])
                x_sb = small.tile([128, R], F32, tag=tag + "x")
                nc.vector.tensor_scalar_mul(x_sb[:], i32x4, rd[:])
                for _ in range(ns_iters):
                    for u in range(4):
                        nc.tensor.matmul(s_t[32 * u:32 * u + 32, 0:32],
                                         v_sb[32 * u:32 * u + 32, :],
                                         x_sb[32 * u:32 * u + 32, :],
                                         start=True, stop=True,
                                         tile_position=(32 * u, 32 * u))
                    y_sb = small.tile([128, R], F32, tag=tag + "y")
                    nc.vector.tensor_sub(y_sb[:], twoI4, s_t[:, 0:32])
                    for u in range(4):
                        nc.tensor.matmul(s_t[32 * u:32 * u + 32, 32:64],
                                         x_sb[32 * u:32 * u + 32, :],
                                         y_sb[32 * u:32 * u + 32, :],
                                         start=True, stop=True,
                                         tile_position=(32 * u, 32 * u))
                    x_sb = small.tile([128, R], F32, tag=tag + "x")
                    nc.scalar.copy(x_sb[:], s_t[:, 32:64])
                return x_sb

            for t in range(n_iters):
                ns_t = psN.tile([128, 512], F32, tag="ns")
                u1 = ps1.tile([128, 512], F32, tag="u1")
                u2 = ps1.tile([128, 512], F32, tag="u2")
                # ---- mode A ----
                gb_sb = grams(ns_t, 0, b4, H, "gbs")
                gc_sb = grams(ns_t, 32, c4, W, "gcs")
                xa = ns_solve(ns_t, gb_sb, gc_sb, "nsa")
                pt4 = ptp.tile([128, JKP], F32, tag="pt4")
                nc.vector.memset(pt4[:, JK:JKP], 0.0)
                nc.vector.tensor_mul(
                    pt4[:, 0:JK].rearrange("p (j k) -> p j k", j=H),
                    bt4[:].unsqueeze(2).broadcast_to([128, H, W]),
                    ct4[:].unsqueeze(1).broadcast_to([128, H, W]))
                for u in range(4):
                    pts = pp_pool.tile([32, JKP], F32, tag="pts")
                    nc.sync.dma_start(pts[:], pt4[32 * u:32 * u + 32, :])
                    p_sb = pp_pool.tile([128, NCHUNK * 32], F32, tag="p_sb")
                    for c0 in range(0, NCHUNK, 16):
                        cs = list(range(c0, min(c0 + 16, NCHUNK)))
                        pp = psT.tile([128, 512], F32, tag="tp")
                        for i, c in enumerate(cs):
                            nc.tensor.transpose(
                                pp[:, 32 * i:32 * i + 32],
                                pts[:, 128 * c:128 * c + 128],
                                i32x4[0:32, :])
                        nc.scalar.copy(p_sb[:, 32 * cs[0]:32 * cs[0] + 32 * len(cs)],
                                       pp[:, 0:32 * len(cs)])
                    for c in range(NCHUNK):
                        nc.tensor.matmul(u1[32 * u:32 * u + 32, 0:128],
                                         p_sb[:, 32 * c:32 * c + 32],
                                         tt[u][:, 128 * c:128 * c + 128],
                                         start=(c == 0), stop=(c == NCHUNK - 1),
                                         tile_position=(0, 32 * u))
                mat_sb = pp_pool.tile([128, 128], F32, tag="mat_sb")
                nc.scalar.copy(mat_sb[:], u1[:, 0:128])
                mat_f = small.tile([32, 512], F32, tag="mat_f")
                xa_f = small.tile([32, 128], F32, tag="xa_f")
                for u in range(4):
                    nc.sync.dma_start(mat_f[:, 128 * u:128 * u + 128],
                                      mat_sb[32 * u:32 * u + 32, :])
                    nc.sync.dma_start(xa_f[:, 32 * u:32 * u + 32],
                                      xa[32 * u:32 * u + 32, :])
                for u in range(4):
                    nc.tensor.matmul(u1[:, 128 + 32 * u:160 + 32 * u],
                                     mat_f[:, 128 * u:128 * u + 128],
                                     xa_f[:, 32 * u:32 * u + 32],
                                     start=True, stop=True)
                a4 = fac.tile([CI, 128], F32, tag="a4")
                nc.scalar.copy(a4[:], u1[:, 128:256])

                # ---- mode B ----
                ga_sb = grams(ns_t, 64, a4, CI, "gas")
                xb = ns_solve(ns_t, ga_sb, gc_sb, "nsb")
                tmpb = big.tile([128, JK], F32, tag="tmpb")
                g_sb = big.tile([128, JK], F32, tag="g_sb")
                off = 0
                for w in GCH:
                    g_ps = psG.tile([128, 512], F32, tag="g")
                    for u in range(4):
                        nc.tensor.matmul(g_ps[32 * u:32 * u + 32, 0:w],
                                         a4[:, 32 * u:32 * u + 32],
                                         tn[u][:, off:off + w],
                                         start=True, stop=True,
                                         tile_position=(0, 32 * u))
                    nj = w // W
                    nc.vector.tensor_mul(
                        tmpb[:, off:off + w].rearrange("p (j k) -> p j k", j=nj),
                        g_ps[:, 0:w].rearrange("p (j k) -> p j k", j=nj),
                        ct4[:].unsqueeze(1).broadcast_to([128, nj, W]))
                    nc.scalar.copy(g_sb[:, off:off + w], g_ps[:, 0:w])
                    off += w
                mbt = small.tile([128, H], F32, tag="mbt")
                roff = 0
                for w in GCH:
                    nj = w // W
                    nc.vector.reduce_sum(
                        mbt[:, roff:roff + nj],
                        tmpb[:, roff * W:roff * W + w].rearrange("p (j k) -> p j k", j=nj),
                        axis=mybir.AxisListType.X)
                    roff += nj
                mbt_f = small.tile([32, 224], F32, tag="mbt_f")
                xb_f = small.tile([32, 128], F32, tag="xb_f")
                for u in range(4):
                    nc.sync.dma_start(mbt_f[:, 56 * u:56 * u + 56],
                                      mbt[32 * u:32 * u + 32, :])
                    nc.sync.dma_start(xb_f[:, 32 * u:32 * u + 32],
                                      xb[32 * u:32 * u + 32, :])
                for u in range(4):
                    nc.tensor.matmul(u1[0:H, 256 + 32 * u:288 + 32 * u],
                                     mbt_f[:, 56 * u:56 * u + 56],
                                     xb_f[:, 32 * u:32 * u + 32],
                                     start=True, stop=True)
                    nc.tensor.matmul(u1[32 * u:32 * u + 32, 384:440],
                                     xb[32 * u:32 * u + 32, :],
                                     mbt[32 * u:32 * u + 32, :],
                                     start=True, stop=True,
                                     tile_position=(32 * u, 32 * u))
                b4 = fac.tile([128, 128], F32, tag="b4")
                bt4 = fac.tile([128, H], F32, tag="bt4")
                nc.vector.memset(b4[:], 0.0)
                nc.scalar.copy(b4[0:H, :], u1[0:H, 256:384])
                nc.scalar.copy(bt4[:], u1[:, 384:440])

                # ---- mode C ----
                gb2_sb = grams(ns_t, 96, b4, H, "gb2s")
                xc = ns_solve(ns_t, ga_sb, gb2_sb, "nsc")
                tmpc = big.tile([128, JK], F32, tag="tmpb", name=f"tmpc_{g}_{t}")
                nc.vector.tensor_mul(
                    tmpc[:].rearrange("p (j k) -> p j k", j=H),
                    g_sb[:].rearrange("p (j k) -> p j k", j=H),
                    bt4[:].unsqueeze(2).broadcast_to([128, H, W]))
                mct = small.tile([128, W], F32, tag="mct")
                nc.vector.reduce_sum(mct[:], tmpc[:].rearrange("p (j k) -> p k j", j=H),
                                     axis=mybir.AxisListType.X)
                mct_f = small.tile([32, 224], F32, tag="mct_f")
                xc_f = small.tile([32, 128], F32, tag="xc_f")
                for u in range(4):
                    nc.sync.dma_start(mct_f[:, 56 * u:56 * u + 56],
                                      mct[32 * u:32 * u + 32, :])
                    nc.sync.dma_start(xc_f[:, 32 * u:32 * u + 32],
                                      xc[32 * u:32 * u + 32, :])
                for u in range(4):
                    nc.tensor.matmul(u2[0:W, 32 * u:32 * u + 32],
                                     mct_f[:, 56 * u:56 * u + 56],
                                     xc_f[:, 32 * u:32 * u + 32],
                                     start=True, stop=True)
                    nc.tensor.matmul(u2[32 * u:32 * u + 32, 128:184],
                                     xc[32 * u:32 * u + 32, :],
                                     mct[32 * u:32 * u + 32, :],
                                     start=True, stop=True,
                                     tile_position=(32 * u, 32 * u))
                c4 = fac.tile([128, 128], F32, tag="c4")
                ct4 = fac.tile([128, W], F32, tag="ct4")
                nc.vector.memset(c4[:], 0.0)
                nc.scalar.copy(c4[0:W, :], u2[0:W, 0:128])
                nc.scalar.copy(ct4[:], u2[:, 128:184])

            # ---- column sums (means before /n) ----
            for u in range(4):
                nc.tensor.matmul(u2[0:R, 184 + 3 * u:185 + 3 * u],
                                 a4[:, 32 * u:32 * u + 32], ones,
                                 start=True, stop=True)
                nc.tensor.matmul(u2[0:R, 185 + 3 * u:186 + 3 * u],
                                 b4[:, 32 * u:32 * u + 32], ones,
                                 start=True, stop=True)
                nc.tensor.matmul(u2[0:R, 186 + 3 * u:187 + 3 * u],
                                 c4[:, 32 * u:32 * u + 32], ones,
                                 start=True, stop=True)
            nc.scalar.copy(out_sb[:, 12 * g:12 * g + 12], u2[0:R, 184:196])
        nc.sync.dma_start(d_out[:], out_sb[:])
    nc.compile()
    return nc


_ENC_C = r"""
#include <stdint.h>
#include <math.h>
void encode24(const float* restrict x, int16_t* restrict hi,
              uint8_t* restrict lo, long n) {
    for (long i = 0; i < n; i++) {
        float y = x[i] * 1048576.0f;
        y = rintf(y);
        if (y > 8388607.0f) y = 8388607.0f;
        if (y < -8388607.0f) y = -8388607.0f;
        int32_t yi = (int32_t)y;
        int32_t h = yi >> 8;
        hi[i] = (int16_t)h;
        lo[i] = (uint8_t)(yi - (h << 8));
    }
}
"""


def _get_encoder():
    """One-pass C encoder (the single-CPU host makes numpy's ~8 passes cost
    >1 s on 205 MB; this is ~10x less memory traffic)."""
    if "enc" in _CACHE:
        return _CACHE["enc"]
    import ctypes, subprocess, tempfile, os
    d = tempfile.mkdtemp()
    src = os.path.join(d, "enc24.c")
    so = os.path.join(d, "enc24.so")
    with open(src, "w") as fh:
        fh.write(_ENC_C)
    subprocess.run(
        ["gcc", "-O3", "-march=native", "-funroll-loops", "-shared", "-fPIC",
         src, "-o", so, "-lm"], check=True, capture_output=True)
    lib = ctypes.CDLL(so)
    lib.encode24.argtypes = [ctypes.c_void_p, ctypes.c_void_p,
                             ctypes.c_void_p, ctypes.c_long]
    _CACHE["enc"] = lib
    return lib


def _np_encode24(xs, xhi, xlo):
    y = xs * S_ENC
    np.rint(y, out=y)
    np.clip(y, -CLIP, CLIP, out=y)
    h = np.floor(y * (1.0 / 256.0))
    xhi[:] = h
    np.multiply(h, 256.0, out=h)
    np.subtract(y, h, out=y)
    xlo[:] = y


def _encode24_chunk(x4, k, lib):
    """Encode chunk k: per-core samples [rows*k, rows*(k+1)) across all cores.
    x4 is x reshaped (BSZ, CI, JK). Returns (hi [8*rows,CI,JK] i16, lo u8)."""
    rows = SPC // NCHK
    xhi = np.empty((NCORES * rows, CI, JK), np.int16)
    xlo = np.empty((NCORES * rows, CI, JK), np.uint8)
    for c in range(NCORES):
        blk = x4[SPC * c + rows * k: SPC * c + rows * (k + 1)]
        dst = slice(rows * c, rows * (c + 1))
        if lib is not None:
            lib.encode24(blk.ctypes.data, xhi[dst].ctypes.data,
                         xlo[dst].ctypes.data, blk.size)
        else:
            _np_encode24(blk, xhi[dst], xlo[dst])
    return xhi, xlo


def _get_runtime():
    if "rt" in _CACHE:
        return _CACHE["rt"]
    import jax
    from jax.sharding import Mesh, PartitionSpec, NamedSharding
    from jax.experimental.shard_map import shard_map
    from concourse.bass2jax import _bass_exec_p, install_neuronx_cc_hook, \
        partition_id_tensor

    nc = _build_program()
    install_neuronx_cc_hook()

    partition_name = None
    pt = getattr(nc, "partition_id_tensor", None)
    if pt is not None:
        partition_name = pt.name

    in_names, out_names, out_avals, zero_shapes = [], [], [], []
    for alloc in nc.m.functions[0].allocations:
        if not isinstance(alloc, mybir.MemoryLocationSet):
            continue
        name = alloc.memorylocations[0].name
        if alloc.kind == "ExternalInput":
            if name != partition_name:
                in_names.append(name)
        elif alloc.kind == "ExternalOutput":
            shape = tuple(alloc.tensor_shape)
            dtype = mybir.dt.np(alloc.dtype)
            out_avals.append(jax.core.ShapedArray(shape, dtype))
            out_names.append(name)
            zero_shapes.append((shape, dtype))
    n_params = len(in_names)
    in_names_all = list(in_names) + list(out_names) + \
        ([partition_name] if partition_name else [])

    def _body(*args):
        operands = list(args)
        if partition_name is not None:
            operands.append(partition_id_tensor())
        outs = _bass_exec_p.bind(
            *operands, out_avals=tuple(out_avals), in_names=tuple(in_names_all),
            out_names=tuple(out_names), lowering_input_output_aliases=(),
            sim_require_finite=True, sim_require_nnan=True, nc=nc)
        return tuple(outs)

    devices = jax.devices()[:NCORES]
    mesh = Mesh(np.asarray(devices), ("core",))
    spec = PartitionSpec("core")
    n_outs = len(out_names)
    sharded = jax.jit(
        shard_map(_body, mesh=mesh, in_specs=(spec,) * (n_params + n_outs),
                  out_specs=(spec,) * n_outs, check_rep=False),
        donate_argnums=tuple(range(n_params, n_params + n_outs)),
        keep_unused=True)
    rt = {
        "nc": nc, "sharded": sharded, "in_names": in_names,
        "out_names": out_names, "zero_shapes": zero_shapes,
        "sharding": NamedSharding(mesh, spec), "jax": jax,
    }
    _CACHE["rt"] = rt
    return rt


def kernel(x, W1, b1, W2, b2, A0, B0, C0, _trace=False):
    rt = _get_runtime()
    jax = rt["jax"]
    x = np.ascontiguousarray(x, dtype=np.float32)
    x4 = x.reshape(BSZ, CI, JK)
    try:
        lib = _get_encoder()
    except Exception:
        lib = None

    # encode chunk k while chunk k-1 is already streaming over the wire
    args = {}
    for k in range(NCHK):
        xhi, xlo = _encode24_chunk(x4, k, lib)
        args[f"xhi{k}"] = jax.device_put(xhi, rt["sharding"])
        args[f"xlo{k}"] = jax.device_put(xlo, rt["sharding"])

    args.update({
        "a0": np.ascontiguousarray(A0, dtype=np.float32),
        "b0": np.ascontiguousarray(B0, dtype=np.float32),
        "c0": np.ascontiguousarray(C0, dtype=np.float32),
    })
    zeros = [np.zeros((NCORES * s[0], *s[1:]), d) for s, d in rt["zero_shapes"]]
    out_arrs = rt["sharded"](*[args[n] for n in rt["in_names"]], *zeros)
    f = np.asarray(out_arrs[rt["out_names"].index("feats")])
    f = f.reshape(NCORES, R, SPC * 3)

    feats = np.empty((BSZ, 3 * R), dtype=np.float32)
    for core in range(NCORES):
        for u in range(SPC):
            s = core * SPC + u
            feats[s, 0:R] = f[core, :, 3 * u] / CI
            feats[s, R:2 * R] = f[core, :, 3 * u + 1] / H
            feats[s, 2 * R:3 * R] = f[core, :, 3 * u + 2] / W

    h = np.maximum(feats @ W1 + b1, 0.0)
    logits = (h @ W2 + b2).astype(np.float32)
    binary_hash = np.sign(logits).astype(np.float32)
    kernel._last_exec_ns = None
    return binary_hash, logits


# revision 23
# speedup vs baseline: 1.0189x; 1.0189x over previous
"""CP-ALS hash layer kernel for Trainium2 (8 NeuronCores, SPMD data-parallel).

Per sample: rank-32 CP-ALS (20 iters) on its (128,56,56) tensor; ridge-regularized
32x32 solves via Newton-Schulz (5 iters, Jacobi diag init); feats -> MLP -> sign
(MLP head on host, fp32). Batch 128 = 16 samples/core, processed in groups of 4
with factor-stacked (4x32=128 partition) DVE ops and tile_position-packed matmuls.

Wall-clock is dominated by shipping inputs over the ~40 MB/s axon tunnel, so:
  - x is sent as 24-bit fixed point (int16 MSB + uint8 LSB = 3 B/elem vs 4) and
    decoded to fp32 on device with exact power-of-two scales (the only loss is
    the initial rint(x * 2^20), ~3e-7 rms -- logits relerr ~2e-6, no sign flips).
  - the jitted shard_map executable is cached across calls (rebuilding it costs
    ~4 s of retrace per call inside run_bass_kernel_spmd's axon path).
  - konst is embedded in the NEFF (Const tensor); b0t/c0t initial factor
    transposes are computed on device instead of being shipped.

PSUM budget (8 banks of 2KB):
  ns   (1): grams gb/gc/ga/gb2 + NS s/xp slices
  u1   (1): M_A^T acc [0:128] | a_ps [128:256] | b_ps [256:384] | bt_ps [384:440]
  u2   (1): c_ps [0:128] | ct_ps [128:184]
  g    (2): G chunk double-buffer
  tp   (2): PE-transpose staging (T^T and P chunks; initial bt/ct transposes)
"""
import sys
sys.path.insert(0, '/opt/trn_rl_repo')
import numpy as np
from contextlib import ExitStack
from concurrent.futures import ThreadPoolExecutor

import concourse.bass as bass
import concourse.tile as tile
from concourse import bacc, mybir

F32 = mybir.dt.float32
I16 = mybir.dt.int16
U8 = mybir.dt.uint8

BSZ, CI, H, W = 128, 128, 56, 56
R = 32
N_ITERS = 20
RIDGE = 1e-6
NCORES = 8
SPC = BSZ // NCORES          # 16 samples per core
JK = H * W                   # 3136
JKP = 3200                   # JK padded to 25*128
NCHUNK = JKP // 128          # 25
GCH = [504] * 6 + [112]      # G chunks at j boundaries (9j*56 ... 2j*56)
NS_ITERS = 5

# 24-bit fixed point encode of x: xi = rint(x * 2^20) clipped to +-(2^23-1),
# hi = floor(xi/256) in int16, lo = xi - 256*hi in uint8.
# Decode: x ~= hi * 2^-12 + lo * 2^-20 (exact fp32 arithmetic).
S_ENC = float(2 ** 20)
CLIP = float(2 ** 23 - 1)
SC_HI = float(2.0 ** -12)
SC_LO = float(2.0 ** -20)
NCHK = 2                     # x ships as 2 combined chunks (bigger transfers
                             # stream faster; 2 keeps encode/wire overlap)
JKH = JK // 2                # 1568
PKW = JK + JKH               # 4704 int16 lanes: hi[0:JK] | packed lo pairs

_CACHE = {}


def _konst_blob():
    k = np.zeros((128, 225), dtype=np.float32)
    k[:, 0:128] = np.eye(128, dtype=np.float32)
    k[:, 128] = 1.0
    i32 = np.eye(R, dtype=np.float32)
    for u in range(4):
        k[32 * u:32 * u + 32, 129:161] = RIDGE * i32
        k[32 * u:32 * u + 32, 161:193] = 2.0 * i32
        k[32 * u:32 * u + 32, 193:225] = i32
    return k


def _build_program(n_groups=SPC // 4, n_iters=N_ITERS, ns_iters=NS_ITERS):
    nc = bacc.Bacc(None, target_bir_lowering=False)
    nsamp = 4 * n_groups

    # x arrives as NCHK chunk pairs (samples [rows*k, rows*(k+1)) per core in
    # chunk k): hi plane int16 + lo plane uint8. Few big transfers stream
    # faster than many small ones; chunk k+1 encodes while chunk k streams.
    d_xhi, d_xlo = [], []
    for k in range(NCHK):
        d_xhi.append(nc.declare_dram_parameter(
            f"xhi{k}", [nsamp // NCHK, CI, JK], I16, isOutput=False))
        d_xlo.append(nc.declare_dram_parameter(
            f"xlo{k}", [nsamp // NCHK, CI, JK], U8, isOutput=False))
    # initial factors A0|B0|C0 stacked on one param: [CI | H | W, R]
    d_fac = nc.declare_dram_parameter("fac", [nsamp, CI + H + W, R], F32,
                                      isOutput=False)
    d_out = nc.declare_dram_parameter("feats", [R, nsamp * 3], F32, isOutput=True)
    d_k = nc.inline_tensor(_konst_blob(), name="konst")
    d_xf = nc.dram_tensor("xf", [nsamp, CI, JK], F32)

    with ExitStack() as ctx:
        tc = ctx.enter_context(tile.TileContext(nc))

        # ---- phase 1: decode int24 -> fp32 scratch in DRAM ----
        rows = nsamp // NCHK
        with tc.tile_pool(name="dec", bufs=2) as dpool:
            for s in range(nsamp):
                hi_sb = dpool.tile([CI, JK], I16, tag="hi")
                lo_sb = dpool.tile([CI, JK], U8, tag="lo")
                nc.sync.dma_start(hi_sb[:], d_xhi[s // rows][s % rows])
                nc.sync.dma_start(lo_sb[:], d_xlo[s // rows][s % rows])
                dec = dpool.tile([CI, JK], F32, tag="dc")
                lo_f = dpool.tile([CI, JK], F32, tag="lf")
                nc.vector.tensor_scalar_mul(dec[:], hi_sb[:], SC_HI)
                nc.vector.tensor_scalar_mul(lo_f[:], lo_sb[:], SC_LO)
                nc.vector.tensor_add(dec[:], dec[:], lo_f[:])
                nc.sync.dma_start(d_xf[s], dec[:])

        konst = ctx.enter_context(tc.tile_pool(name="konst", bufs=1))
        tn_pool = ctx.enter_context(tc.tile_pool(name="tn", bufs=4))
        tt_pool = ctx.enter_context(tc.tile_pool(name="tt", bufs=4))
        small = ctx.enter_context(tc.tile_pool(name="small", bufs=2))
        fac = ctx.enter_context(tc.tile_pool(name="fac", bufs=2))
        big = ctx.enter_context(tc.tile_pool(name="big", bufs=1))
        pp_pool = ctx.enter_context(tc.tile_pool(name="ppool", bufs=2))
        ps1 = ctx.enter_context(tc.tile_pool(name="ps1", bufs=1, space="PSUM"))
        psN = ctx.enter_context(tc.tile_pool(name="psN", bufs=1, space="PSUM"))
        psG = ctx.enter_context(tc.tile_pool(name="psG", bufs=2, space="PSUM"))
        psT = ctx.enter_context(tc.tile_pool(name="psT", bufs=2, space="PSUM"))
        ptp = ctx.enter_context(tc.tile_pool(name="ptp", bufs=2))
        out_pool = ctx.enter_context(tc.tile_pool(name="outp", bufs=1))

        k_sb = konst.tile([128, 225], F32)
        nc.sync.dma_start(k_sb[:], d_k[:])
        ident = k_sb[:, 0:128]
        ones = k_sb[:, 128:129]
        twoI4 = k_sb[:, 161:193]
        i32x4 = k_sb[:, 193:225]

        out_sb = out_pool.tile([R, nsamp * 3], F32)

        for g in range(n_groups):
            # ---- load tensor + transpose copies ----
            tn = [tn_pool.tile([CI, JKP], F32, tag="tn", name=f"tn{g}_{u}") for u in range(4)]
            tt = [tt_pool.tile([128, JKP], F32, tag="tt", name=f"tt{g}_{u}") for u in range(4)]
            for u in range(4):
                nc.sync.dma_start(tn[u][:, 0:JK], d_xf[4 * g + u])
                nc.vector.memset(tn[u][:, JK:JKP], 0.0)
            for u in range(4):
                for c0 in range(0, NCHUNK, 4):
                    cs = list(range(c0, min(c0 + 4, NCHUNK)))
                    tp_ps = psT.tile([128, 512], F32, tag="tp")
                    for i, c in enumerate(cs):
                        nc.tensor.transpose(tp_ps[:, 128 * i:128 * i + 128],
                                            tn[u][:, 128 * c:128 * c + 128], ident)
                    nc.scalar.copy(tt[u][:, 128 * cs[0]:128 * cs[0] + 128 * len(cs)],
                                   tp_ps[:, 0:128 * len(cs)])

            # ---- factors ----
            a4 = fac.tile([CI, 128], F32, tag="a4")
            b4 = fac.tile([128, 128], F32, tag="b4")
            c4 = fac.tile([128, 128], F32, tag="c4")
            bt4 = fac.tile([128, H], F32, tag="bt4")
            ct4 = fac.tile([128, W], F32, tag="ct4")
            nc.vector.memset(b4[:], 0.0)
            nc.vector.memset(c4[:], 0.0)
            for u in range(4):
                nc.sync.dma_start(a4[:, 32 * u:32 * u + 32],
                                  d_fac[4 * g + u, 0:CI])
                nc.sync.dma_start(b4[0:H, 32 * u:32 * u + 32],
                                  d_fac[4 * g + u, CI:CI + H])
                nc.sync.dma_start(c4[0:W, 32 * u:32 * u + 32],
                                  d_fac[4 * g + u, CI + H:CI + H + W])
            # initial bt4/ct4 = b4^T[:, :H], c4^T[:, :W] via PE transpose
            tp_ps = psT.tile([128, 512], F32, tag="tp", name=f"bt0_{g}")
            nc.tensor.transpose(tp_ps[:, 0:128], b4[:], ident)
            nc.tensor.transpose(tp_ps[:, 128:256], c4[:], ident)
            nc.scalar.copy(bt4[:], tp_ps[:, 0:H])
            nc.scalar.copy(ct4[:], tp_ps[:, 128:128 + W])

            def grams(ns_t, col, mat, np_, tag):
                for u in range(4):
                    nc.tensor.matmul(ns_t[32 * u:32 * u + 32, col:col + 32],
                                     mat[:, 32 * u:32 * u + 32],
                                     mat[:, 32 * u:32 * u + 32],
                                     start=True, stop=True, tile_position=(0, 32 * u))
                g_sb = small.tile([128, R], F32, tag=tag, name="gr_" + tag)
                nc.scalar.copy(g_sb[:], ns_t[:, col:col + 32])
                return g_sb

            def ns_solve(ns_t, gx_sb, gy_sb, tag):
                s_t = psN.tile([128, 64], F32, tag="nss", name="nss_" + tag)
                v_sb = small.tile([128, R], F32, tag=tag + "v")
                nc.vector.tensor_mul(v_sb[:], gx_sb[:], gy_sb[:])
                dm = small.tile([128, R], F32, tag=tag + "dm")
                nc.vector.tensor_mul(dm[:], v_sb[:], i32x4)
                dcol = small.tile([128, 1], F32, tag=tag + "dc")
                nc.vector.reduce_sum(dcol[:], dm[:], axis=mybir.AxisListType.X)
                rd = small.tile([128, 1], F32, tag=tag + "rd")
                nc.vector.reciprocal(rd[:], dcol[:])
                x_sb = small.tile([128, R], F32, tag=tag + "x")
                nc.vector.tensor_scalar_mul(x_sb[:], i32x4, rd[:])
                for _ in range(ns_iters):
                    for u in range(4):
                        nc.tensor.matmul(s_t[32 * u:32 * u + 32, 0:32],
                                         v_sb[32 * u:32 * u + 32, :],
                                         x_sb[32 * u:32 * u + 32, :],
                                         start=True, stop=True,
                                         tile_position=(32 * u, 32 * u))
                    y_sb = small.tile([128, R], F32, tag=tag + "y")
                    nc.vector.tensor_sub(y_sb[:], twoI4, s_t[:, 0:32])
                    for u in range(4):
                        nc.tensor.matmul(s_t[32 * u:32 * u + 32, 32:64],
                                         x_sb[32 * u:32 * u + 32, :],
                                         y_sb[32 * u:32 * u + 32, :],
                                         start=True, stop=True,
                                         tile_position=(32 * u, 32 * u))
                    x_sb = small.tile([128, R], F32, tag=tag + "x")
                    nc.scalar.copy(x_sb[:], s_t[:, 32:64])
                return x_sb

            for t in range(n_iters):
                ns_t = psN.tile([128, 512], F32, tag="ns")
                u1 = ps1.tile([128, 512], F32, tag="u1")
                u2 = ps1.tile([128, 512], F32, tag="u2")
                # ---- mode A ----
                gb_sb = grams(ns_t, 0, b4, H, "gbs")
                gc_sb = grams(ns_t, 32, c4, W, "gcs")
                xa = ns_solve(ns_t, gb_sb, gc_sb, "nsa")
                pt4 = ptp.tile([128, JKP], F32, tag="pt4")
                nc.vector.memset(pt4[:, JK:JKP], 0.0)
                nc.vector.tensor_mul(
                    pt4[:, 0:JK].rearrange("p (j k) -> p j k", j=H),
                    bt4[:].unsqueeze(2).broadcast_to([128, H, W]),
                    ct4[:].unsqueeze(1).broadcast_to([128, H, W]))
                for u in range(4):
                    pts = pp_pool.tile([32, JKP], F32, tag="pts")
                    nc.sync.dma_start(pts[:], pt4[32 * u:32 * u + 32, :])
                    p_sb = pp_pool.tile([128, NCHUNK * 32], F32, tag="p_sb")
                    for c0 in range(0, NCHUNK, 16):
                        cs = list(range(c0, min(c0 + 16, NCHUNK)))
                        pp = psT.tile([128, 512], F32, tag="tp")
                        for i, c in enumerate(cs):
                            nc.tensor.transpose(
                                pp[:, 32 * i:32 * i + 32],
                                pts[:, 128 * c:128 * c + 128],
                                i32x4[0:32, :])
                        nc.scalar.copy(p_sb[:, 32 * cs[0]:32 * cs[0] + 32 * len(cs)],
                                       pp[:, 0:32 * len(cs)])
                    for c in range(NCHUNK):
                        nc.tensor.matmul(u1[32 * u:32 * u + 32, 0:128],
                                         p_sb[:, 32 * c:32 * c + 32],
                                         tt[u][:, 128 * c:128 * c + 128],
                                         start=(c == 0), stop=(c == NCHUNK - 1),
                                         tile_position=(0, 32 * u))
                mat_sb = pp_pool.tile([128, 128], F32, tag="mat_sb")
                nc.scalar.copy(mat_sb[:], u1[:, 0:128])
                mat_f = small.tile([32, 512], F32, tag="mat_f")
                xa_f = small.tile([32, 128], F32, tag="xa_f")
                for u in range(4):
                    nc.sync.dma_start(mat_f[:, 128 * u:128 * u + 128],
                                      mat_sb[32 * u:32 * u + 32, :])
                    nc.sync.dma_start(xa_f[:, 32 * u:32 * u + 32],
                                      xa[32 * u:32 * u + 32, :])
                for u in range(4):
                    nc.tensor.matmul(u1[:, 128 + 32 * u:160 + 32 * u],
                                     mat_f[:, 128 * u:128 * u + 128],
                                     xa_f[:, 32 * u:32 * u + 32],
                                     start=True, stop=True)
                a4 = fac.tile([CI, 128], F32, tag="a4")
                nc.scalar.copy(a4[:], u1[:, 128:256])

                # ---- mode B ----
                ga_sb = grams(ns_t, 64, a4, CI, "gas")
                xb = ns_solve(ns_t, ga_sb, gc_sb, "nsb")
                tmpb = big.tile([128, JK], F32, tag="tmpb")
                g_sb = big.tile([128, JK], F32, tag="g_sb")
                off = 0
                for w in GCH:
                    g_ps = psG.tile([128, 512], F32, tag="g")
                    for u in range(4):
                        nc.tensor.matmul(g_ps[32 * u:32 * u + 32, 0:w],
                                         a4[:, 32 * u:32 * u + 32],
                                         tn[u][:, off:off + w],
                                         start=True, stop=True,
                                         tile_position=(0, 32 * u))
                    nj = w // W
                    nc.vector.tensor_mul(
                        tmpb[:, off:off + w].rearrange("p (j k) -> p j k", j=nj),
                        g_ps[:, 0:w].rearrange("p (j k) -> p j k", j=nj),
                        ct4[:].unsqueeze(1).broadcast_to([128, nj, W]))
                    nc.scalar.copy(g_sb[:, off:off + w], g_ps[:, 0:w])
                    off += w
                mbt = small.tile([128, H], F32, tag="mbt")
                roff = 0
                for w in GCH:
                    nj = w // W
                    nc.vector.reduce_sum(
                        mbt[:, roff:roff + nj],
                        tmpb[:, roff * W:roff * W + w].rearrange("p (j k) -> p j k", j=nj),
                        axis=mybir.AxisListType.X)
                    roff += nj
                mbt_f = small.tile([32, 224], F32, tag="mbt_f")
                xb_f = small.tile([32, 128], F32, tag="xb_f")
                for u in range(4):
                    nc.sync.dma_start(mbt_f[:, 56 * u:56 * u + 56],
                                      mbt[32 * u:32 * u + 32, :])
                    nc.sync.dma_start(xb_f[:, 32 * u:32 * u + 32],
                                      xb[32 * u:32 * u + 32, :])
                for u in range(4):
                    nc.tensor.matmul(u1[0:H, 256 + 32 * u:288 + 32 * u],
                                     mbt_f[:, 56 * u:56 * u + 56],
                                     xb_f[:, 32 * u:32 * u + 32],
                                     start=True, stop=True)
                    nc.tensor.matmul(u1[32 * u:32 * u + 32, 384:440],
                                     xb[32 * u:32 * u + 32, :],
                                     mbt[32 * u:32 * u + 32, :],
                                     start=True, stop=True,
                                     tile_position=(32 * u, 32 * u))
                b4 = fac.tile([128, 128], F32, tag="b4")
                bt4 = fac.tile([128, H], F32, tag="bt4")
                nc.vector.memset(b4[:], 0.0)
                nc.scalar.copy(b4[0:H, :], u1[0:H, 256:384])
                nc.scalar.copy(bt4[:], u1[:, 384:440])

                # ---- mode C ----
                gb2_sb = grams(ns_t, 96, b4, H, "gb2s")
                xc = ns_solve(ns_t, ga_sb, gb2_sb, "nsc")
                tmpc = big.tile([128, JK], F32, tag="tmpb", name=f"tmpc_{g}_{t}")
                nc.vector.tensor_mul(
                    tmpc[:].rearrange("p (j k) -> p j k", j=H),
                    g_sb[:].rearrange("p (j k) -> p j k", j=H),
                    bt4[:].unsqueeze(2).broadcast_to([128, H, W]))
                mct = small.tile([128, W], F32, tag="mct")
                nc.vector.reduce_sum(mct[:], tmpc[:].rearrange("p (j k) -> p k j", j=H),
                                     axis=mybir.AxisListType.X)
                mct_f = small.tile([32, 224], F32, tag="mct_f")
                xc_f = small.tile([32, 128], F32, tag="xc_f")
                for u in range(4):
                    nc.sync.dma_start(mct_f[:, 56 * u:56 * u + 56],
                                      mct[32 * u:32 * u + 32, :])
                    nc.sync.dma_start(xc_f[:, 32 * u:32 * u + 32],
                                      xc[32 * u:32 * u + 32, :])
                for u in range(4):
                    nc.tensor.matmul(u2[0:W, 32 * u:32 * u + 32],
                                     mct_f[:, 56 * u:56 * u + 56],
                                     xc_f[:, 32 * u:32 * u + 32],
                                     start=True, stop=True)
                    nc.tensor.matmul(u2[32 * u:32 * u + 32, 128:184],
                                     xc[32 * u:32 * u + 32, :],
                                     mct[32 * u:32 * u + 32, :],
                                     start=True, stop=True,
                                     tile_position=(32 * u, 32 * u))
                c4 = fac.tile([128, 128], F32, tag="c4")
                ct4 = fac.tile([128, W], F32, tag="ct4")
                nc.vector.memset(c4[:], 0.0)
                nc.scalar.copy(c4[0:W, :], u2[0:W, 0:128])
                nc.scalar.copy(ct4[:], u2[:, 128:184])

            # ---- column sums (means before /n) ----
            for u in range(4):
                nc.tensor.matmul(u2[0:R, 184 + 3 * u:185 + 3 * u],
                                 a4[:, 32 * u:32 * u + 32], ones,
                                 start=True, stop=True)
                nc.tensor.matmul(u2[0:R, 185 + 3 * u:186 + 3 * u],
                                 b4[:, 32 * u:32 * u + 32], ones,
                                 start=True, stop=True)
                nc.tensor.matmul(u2[0:R, 186 + 3 * u:187 + 3 * u],
                                 c4[:, 32 * u:32 * u + 32], ones,
                                 start=True, stop=True)
            nc.scalar.copy(out_sb[:, 12 * g:12 * g + 12], u2[0:R, 184:196])
        nc.sync.dma_start(d_out[:], out_sb[:])
    nc.compile()
    return nc


_ENC_C = r"""
#include <stdint.h>
#include <math.h>
void encode24(const float* restrict x, int16_t* restrict hi,
              uint8_t* restrict lo, long n) {
    for (long i = 0; i < n; i++) {
        float y = x[i] * 1048576.0f;
        y = rintf(y);
        if (y > 8388607.0f) y = 8388607.0f;
        if (y < -8388607.0f) y = -8388607.0f;
        int32_t yi = (int32_t)y;
        int32_t h = yi >> 8;
        hi[i] = (int16_t)h;
        lo[i] = (uint8_t)(yi - (h << 8));
    }
}
"""


def _get_encoder():
    """One-pass C encoder (the single-CPU host makes numpy's ~8 passes cost
    >1 s on 205 MB; this is ~10x less memory traffic)."""
    if "enc" in _CACHE:
        return _CACHE["enc"]
    import ctypes, subprocess, tempfile, os
    d = tempfile.mkdtemp()
    src = os.path.join(d, "enc24.c")
    so = os.path.join(d, "enc24.so")
    with open(src, "w") as fh:
        fh.write(_ENC_C)
    subprocess.run(
        ["gcc", "-O3", "-march=native", "-funroll-loops", "-shared", "-fPIC",
         src, "-o", so, "-lm"], check=True, capture_output=True)
    lib = ctypes.CDLL(so)
    lib.encode24.argtypes = [ctypes.c_void_p, ctypes.c_void_p,
                             ctypes.c_void_p, ctypes.c_long]
    _CACHE["enc"] = lib
    return lib


def _np_encode24(xs, xhi, xlo):
    y = xs * S_ENC
    np.rint(y, out=y)
    np.clip(y, -CLIP, CLIP, out=y)
    h = np.floor(y * (1.0 / 256.0))
    xhi[:] = h
    np.multiply(h, 256.0, out=h)
    np.subtract(y, h, out=y)
    xlo[:] = y


def _encode24_chunk(x4, k, lib):
    """Encode chunk k: per-core samples [rows*k, rows*(k+1)) across all cores.
    x4 is x reshaped (BSZ, CI, JK). Returns (hi int16, lo uint8)."""
    rows = SPC // NCHK
    xhi = np.empty((NCORES * rows, CI, JK), np.int16)
    xlo = np.empty((NCORES * rows, CI, JK), np.uint8)
    for c in range(NCORES):
        blk = x4[SPC * c + rows * k: SPC * c + rows * (k + 1)]
        dst = slice(rows * c, rows * (c + 1))
        if lib is not None:
            lib.encode24(blk.ctypes.data, xhi[dst].ctypes.data,
                         xlo[dst].ctypes.data, blk.size)
        else:
            _np_encode24(blk, xhi[dst], xlo[dst])
    return xhi, xlo


def _get_runtime():
    if "rt" in _CACHE:
        return _CACHE["rt"]
    import jax
    from jax.sharding import Mesh, PartitionSpec, NamedSharding
    from jax.experimental.shard_map import shard_map
    from concourse.bass2jax import _bass_exec_p, install_neuronx_cc_hook, \
        partition_id_tensor

    nc = _build_program()
    install_neuronx_cc_hook()

    partition_name = None
    pt = getattr(nc, "partition_id_tensor", None)
    if pt is not None:
        partition_name = pt.name

    in_names, out_names, out_avals, zero_shapes = [], [], [], []
    for alloc in nc.m.functions[0].allocations:
        if not isinstance(alloc, mybir.MemoryLocationSet):
            continue
        name = alloc.memorylocations[0].name
        if alloc.kind == "ExternalInput":
            if name != partition_name:
                in_names.append(name)
        elif alloc.kind == "ExternalOutput":
            shape = tuple(alloc.tensor_shape)
            dtype = mybir.dt.np(alloc.dtype)
            out_avals.append(jax.core.ShapedArray(shape, dtype))
            out_names.append(name)
            zero_shapes.append((shape, dtype))
    n_params = len(in_names)
    in_names_all = list(in_names) + list(out_names) + \
        ([partition_name] if partition_name else [])

    def _body(*args):
        operands = list(args)
        if partition_name is not None:
            operands.append(partition_id_tensor())
        outs = _bass_exec_p.bind(
            *operands, out_avals=tuple(out_avals), in_names=tuple(in_names_all),
            out_names=tuple(out_names), lowering_input_output_aliases=(),
            sim_require_finite=True, sim_require_nnan=True, nc=nc)
        return tuple(outs)

    devices = jax.devices()[:NCORES]
    mesh = Mesh(np.asarray(devices), ("core",))
    spec = PartitionSpec("core")
    n_outs = len(out_names)
    sharded = jax.jit(
        shard_map(_body, mesh=mesh, in_specs=(spec,) * (n_params + n_outs),
                  out_specs=(spec,) * n_outs, check_rep=False),
        donate_argnums=tuple(range(n_params, n_params + n_outs)),
        keep_unused=True)
    rt = {
        "nc": nc, "sharded": sharded, "in_names": in_names,
        "out_names": out_names, "zero_shapes": zero_shapes,
        "sharding": NamedSharding(mesh, spec), "jax": jax,
    }
    _CACHE["rt"] = rt
    return rt


def kernel(x, W1, b1, W2, b2, A0, B0, C0, _trace=False):
    rt = _get_runtime()
    jax = rt["jax"]
    x = np.ascontiguousarray(x, dtype=np.float32)
    x4 = x.reshape(BSZ, CI, JK)
    try:
        lib = _get_encoder()
    except Exception:
        lib = None

    # encode chunk k while chunk k-1 is already streaming over the wire
    args = {}
    for k in range(NCHK):
        xhi, xlo = _encode24_chunk(x4, k, lib)
        args[f"xhi{k}"] = jax.device_put(xhi, rt["sharding"])
        args[f"xlo{k}"] = jax.device_put(xlo, rt["sharding"])

    fac = np.empty((BSZ, CI + H + W, R), np.float32)
    fac[:, 0:CI] = A0
    fac[:, CI:CI + H] = B0
    fac[:, CI + H:] = C0
    args["fac"] = fac
    zeros = [np.zeros((NCORES * s[0], *s[1:]), d) for s, d in rt["zero_shapes"]]
    out_arrs = rt["sharded"](*[args[n] for n in rt["in_names"]], *zeros)
    f = np.asarray(out_arrs[rt["out_names"].index("feats")])
    f = f.reshape(NCORES, R, SPC * 3)

    feats = np.empty((BSZ, 3 * R), dtype=np.float32)
    for core in range(NCORES):
        for u in range(SPC):
            s = core * SPC + u
            feats[s, 0:R] = f[core, :, 3 * u] / CI
            feats[s, R:2 * R] = f[core, :, 3 * u + 1] / H
            feats[s, 2 * R:3 * R] = f[core, :, 3 * u + 2] / W

    h = np.maximum(feats @ W1 + b1, 0.0)
    logits = (h @ W2 + b2).astype(np.float32)
    binary_hash = np.sign(logits).astype(np.float32)
    kernel._last_exec_ns = None
    return binary_hash, logits
